# revision 1
# baseline (speedup 1.0000x reference)
"""CenterFormer bbox head as a fused 3-stage matmul chain on 8 TRN2 cores.

Reference computation (per batch b, per proposal n):
  y = relu(BN(shared_w @ x + shared_b))            # 256 -> 64
  h = relu(BN(heads_w1[h] @ y + heads_b1[h]))      # 64 -> 64, 6 heads
  o = heads_w2[h] @ h + heads_b2[h]                # 64 -> 3 (padded), slice+concat -> 12

Host-side preprocessing folds BN (eval mode) into the conv weights, stacks the
6 head convs into a single [384, 64] matmul, and builds a block-diagonal
[12, 384] final conv that directly emits the channel-concatenated output.

Sharding: data-parallel over batch: core b handles ct_feat[b] ([256, 16384]).

All constants are packed into two tensors loaded with one DMA each (weights in
the matmul dtype, biases in f32), and tiny warm-up ops make PE/ACT observe
those DMAs up front: a self-loading fp32 Matmult only has one sync-wait slot
in walrus codegen, so no matmul may ever need to wait on two semaphores.
"""

import numpy as np

BN_EPS = 1e-3
HEAD_CH = (3, 2, 1, 3, 2, 1)
B, CIN, N, CS, HN = 8, 256, 16384, 64, 6
COUT = sum(HEAD_CH)  # 12
NCORES = 8

# matmul dtype: f32r = fp32 bits streamed in float32r PE mode: full matmul
# rate (4x faster than plain f32 on TRN2), measured rel err ~2.7e-4
MM_DTYPE = "f32r"

F = 512    # matmul free-dim tile (one fp32 PSUM bank)
FD = 4096  # staging width; input DMAs issue per 512-col chunk

# packed weight-tile column offsets: w1 [128,128] | w2 [64,384] | w3 [128,36]
W1_OFF, W2_OFF, W3_OFF, W_COLS = 0, 128, 512, 548
# packed bias-tile column offsets: b1 [64,1] | b2 [128,3] | b3 [12,1]
B1_OFF, B2_OFF, B3_OFF, B_COLS = 0, 1, 4, 5

_CACHE: dict = {}


def _build_bass(mm_dtype: str, repeat: int = 1):
    import concourse.bacc as bacc
    import concourse.mybir as mybir
    from concourse.tile import TileContext

    f32 = mybir.dt.float32
    # f32r: stream fp32 bits through the PE in float32r mode (full rate at
    # free-dim >= 256, vs 4 cycles/row for plain fp32). Same 4-byte layout;
    # matmul operands and their producers carry the float32r dtype.
    mdt = {"f32": f32, "f32r": mybir.dt.float32r, "bf16": mybir.dt.bfloat16,
           "f16": mybir.dt.float16}[mm_dtype]
    AF = mybir.ActivationFunctionType
    r = lambda ap: ap
    # x input tiles carry the matmul dtype directly in f32r mode (no cast)
    xdt = mdt if mm_dtype == "f32r" else f32

    # Bacc (not raw Bass): its finalize() runs move_matmul_waits_to_ldweights
    # + generate_event_semaphores, which split multi-sem waits that walrus
    # codegen rejects ("Too many sync wait commands").
    nc = bacc.Bacc()
    x = nc.declare_dram_parameter("x", [CIN, N], xdt, isOutput=False)
    wp = nc.declare_dram_parameter("wp", [128, W_COLS], mdt, isOutput=False)
    bp = nc.declare_dram_parameter("bp", [128, B_COLS], f32, isOutput=False)
    out = nc.declare_dram_parameter("out", [COUT, N], f32, isOutput=True)

    with TileContext(nc) as tc:
        with (
            tc.tile_pool(name="const", bufs=1) as cpool,
            tc.tile_pool(name="xin", bufs=3) as xpool,
            tc.tile_pool(name="acts", bufs=8) as apool,
            tc.tile_pool(name="outs", bufs=4) as opool,
            tc.tile_pool(name="psum", bufs=2, space="PSUM") as ppool,
        ):
            wt = cpool.tile([128, W_COLS], mdt)
            nc.scalar.dma_start(out=wt[:], in_=wp[:])
            bt = cpool.tile([128, B_COLS], f32)
            nc.scalar.dma_start(out=bt[:], in_=bp[:])

            w1 = wt[:, W1_OFF : W1_OFF + 128]          # stage-1 lhsT, 2 K-chunks
            w2 = wt[:64, W2_OFF : W2_OFF + 384]        # stage-2 lhsT
            w3 = wt[:, W3_OFF : W3_OFF + 36]           # stage-3 lhsT, 3 K-chunks
            b1 = bt[:CS, B1_OFF : B1_OFF + 1]
            b3 = bt[:COUT, B3_OFF : B3_OFF + 1]

            # Warm-ups: make PE/ACT observe the const DMAs via single-wait ops
            # so no later matmul needs a second sync-wait slot.
            pw = ppool.tile([1, 1], f32, tag="po")
            wwu = (wt[:, 0:1].bitcast(f32) if mm_dtype == "f32r"
                   else wt[:, 0:1])
            nc.tensor.matmul(pw[:], wwu, wwu, start=True, stop=True)
            sw = apool.tile([1, 1], f32, tag="warm")
            nc.scalar.activation(sw[:], bt[0:1, 0:1], AF.Copy)

            xr = x.rearrange("(k p) n -> p k n", p=128)

            # benchmarking: wrap the whole pass in a HW loop (repeat > 1)
            import contextlib
            loop_cm = (tc.For_i(0, repeat,
                                hint_engines=(mybir.EngineType.PE,))
                       if repeat > 1 else contextlib.nullcontext())

            # Greedy elementwise load-balancer across ACT / DVE / Pool.
            # Costs (ns, from the cost model at F=512): activation 612,
            # tensor_scalar 658, cast [128,2,512]: DVE 594 / Pool 1517.
            est = {"ACT": 0.0, "DVE": 0.0, "POOL": 0.0}

            def relu_bias(dst, src, bias_ap):
                if est["ACT"] + 612 <= est["DVE"] + 658:
                    est["ACT"] += 612
                    nc.scalar.activation(dst, src, AF.Relu, bias=bias_ap)
                else:
                    est["DVE"] += 658
                    nc.vector.tensor_scalar(dst, src, bias_ap, 0.0,
                                            mybir.AluOpType.add,
                                            mybir.AluOpType.max)

            def add_bias(dst, src, bias_ap):
                if est["ACT"] + 612 <= est["DVE"] + 658:
                    est["ACT"] += 612
                    nc.scalar.activation(dst, src, AF.Identity, bias=bias_ap)
                else:
                    est["DVE"] += 658
                    nc.vector.tensor_scalar(dst, src, bias_ap, None,
                                            mybir.AluOpType.add)

            def cast(dst, src):
                c = {"ACT": 1224, "DVE": 594, "POOL": 1517}
                eng = min(est, key=lambda e: est[e] + c[e])
                est[eng] += c[eng]
                if eng == "ACT":
                    nc.scalar.activation(dst, src, AF.Copy)
                elif eng == "DVE":
                    nc.vector.tensor_copy(dst, src)
                else:
                    nc.gpsimd.tensor_copy(dst, src)

            with loop_cm:
              for i in range(N // FD):
                xt = xpool.tile([128, 2, FD], xdt, tag="xt")
                for j in range(FD // F):
                    nc.sync.dma_start(
                        out=xt[:, :, j * F : (j + 1) * F],
                        in_=xr[:, :, i * FD + j * F : i * FD + (j + 1) * F])
                if mm_dtype in ("f32", "f32r"):
                    xm = xt
                else:
                    # split per j-tile so the first matmuls start sooner
                    xm = xpool.tile([128, 2, FD], mdt, tag="xm")
                    for j in range(FD // F):
                        jsl = slice(j * F, (j + 1) * F)
                        cast(xm[:, :, jsl], xt[:, :, jsl])
                ot = opool.tile([COUT, FD], f32, tag="ot")
                for j in range(FD // F):
                    sl = slice(j * F, (j + 1) * F)
                    py = ppool.tile([64, F], f32, tag="py")
                    nc.tensor.matmul(py[:], r(w1[:, 0:64]), r(xm[:, 0, sl]),
                                     start=True, stop=False)
                    nc.tensor.matmul(py[:], r(w1[:, 64:128]), r(xm[:, 1, sl]),
                                     start=False, stop=True)
                    ys = apool.tile([64, F], mdt, tag="ys")
                    relu_bias(ys[:], py[:], b1)
                    po = ppool.tile([COUT, F], f32, tag="po")
                    hss = []
                    for m in range(3):
                        ph = ppool.tile([128, F], f32, tag="ph", bufs=4)
                        nc.tensor.matmul(ph[:], r(w2[:, m * 128 : (m + 1) * 128]),
                                         r(ys[:]), start=True, stop=True)
                        hs = apool.tile([128, F], mdt, tag="hs")
                        relu_bias(hs[:], ph[:],
                                  bt[:, B2_OFF + m : B2_OFF + m + 1])
                        hss.append(hs)
                    # stage-3 matmuls after the hs loop: shortens the po
                    # psum-tile lifetime so more j-tiles pipeline
                    for m in range(3):
                        nc.tensor.matmul(po[:], r(w3[:, m * 12 : (m + 1) * 12]),
                                         r(hss[m][:]), start=(m == 0),
                                         stop=(m == 2))
                    add_bias(ot[:, sl], po[:], b3)
                nc.scalar.dma_start(out=out[:, i * FD : (i + 1) * FD], in_=ot[:])

    nc.finalize()  # runs Bacc.compile(): wait-splitting, reg-alloc, DCE
    _check_matmul_waits(nc)
    return nc


def _check_matmul_waits(nc):
    import concourse.mybir as mybir

    bad = []
    for f in nc.m.functions:
        for blk in f.blocks:
            for inst in blk.instructions:
                if isinstance(inst, mybir.InstMatmult) and inst.sync_info:
                    if len(inst.sync_info.on_wait) > 1:
                        bad.append((inst.name,
                                    [w.ant_name for w in inst.sync_info.on_wait]))
    if bad:
        raise RuntimeError(f"matmuls with >1 sync wait (walrus limit): {bad}")


def _get_nc(mm_dtype: str, repeat: int = 1):
    key = (mm_dtype, repeat)
    if key not in _CACHE:
        _CACHE[key] = _build_bass(mm_dtype, repeat)
    return _CACHE[key]


def _fold_params(inputs, mm_dtype: str):
    """Fold BN into conv weights; pack into the on-device tile layouts."""
    f = lambda k: np.asarray(inputs[k], np.float32)

    inv1 = f("shared_gamma") / np.sqrt(f("shared_var") + BN_EPS)          # [64]
    W1 = f("shared_w") * inv1[:, None]                                    # [64, 256]
    b1v = f("shared_b") * inv1 + f("shared_beta") - f("shared_mean") * inv1

    inv2 = f("heads_gamma") / np.sqrt(f("heads_var") + BN_EPS)            # [6, 64]
    W2 = (f("heads_w1") * inv2[:, :, None]).reshape(HN * CS, CS)          # [384, 64]
    b2v = (f("heads_b1") * inv2 + f("heads_beta")
           - f("heads_mean") * inv2).reshape(HN * CS)                     # [384]

    hw2, hb2 = f("heads_w2"), f("heads_b2")
    W3 = np.zeros((COUT, HN * CS), np.float32)                            # [12, 384]
    b3v = np.zeros((COUT,), np.float32)
    r = 0
    for h, ch in enumerate(HEAD_CH):
        W3[r : r + ch, h * CS : (h + 1) * CS] = hw2[h, :ch, :]
        b3v[r : r + ch] = hb2[h, :ch]
        r += ch

    # lhsT packings (lhsT = W.T, K-chunks of 128 side by side in the free dim)
    wp = np.zeros((128, W_COLS), np.float32)
    wp[:, W1_OFF : W1_OFF + 128] = (
        W1.T.reshape(2, 128, 64).transpose(1, 0, 2).reshape(128, 128))
    wp[:64, W2_OFF : W2_OFF + 384] = W2.T
    wp[:, W3_OFF : W3_OFF + 36] = (
        W3.T.reshape(3, 128, COUT).transpose(1, 0, 2).reshape(128, 36))

    bpk = np.zeros((128, B_COLS), np.float32)
    bpk[:CS, B1_OFF] = b1v
    bpk[:, B2_OFF : B2_OFF + 3] = b2v.reshape(3, 128).T
    bpk[:COUT, B3_OFF] = b3v

    if mm_dtype == "bf16":
        import ml_dtypes
        wp = wp.astype(ml_dtypes.bfloat16)
    elif mm_dtype == "f16":
        wp = wp.astype(np.float16)

    return {"wp": wp, "bp": bpk}, b3v


def _run(inputs, mm_dtype=MM_DTYPE, trace=False):
    from concourse.bass_utils import run_bass_kernel_spmd

    nc = _get_nc(mm_dtype)
    shared, b3v = _fold_params(inputs, mm_dtype)
    ct = np.asarray(inputs["ct_feat"], np.float32)
    in_maps = [
        {"x": np.ascontiguousarray(ct[b]), **shared} for b in range(B)
    ]
    res = run_bass_kernel_spmd(nc, in_maps, core_ids=list(range(NCORES)),
                               trace=trace)
    out = np.stack([res.results[b]["out"] for b in range(B)], axis=0)
    return out, res


def kernel(**inputs) -> np.ndarray:
    out, _ = _run(inputs)
    return out



# revision 14
# speedup vs baseline: 1.4491x; 1.4491x over previous
"""CenterFormer bbox head as a fused 3-stage matmul chain on 8 TRN2 cores.

Reference computation (per batch b, per proposal n):
  y = relu(BN(shared_w @ x + shared_b))            # 256 -> 64
  h = relu(BN(heads_w1[h] @ y + heads_b1[h]))      # 64 -> 64, 6 heads
  o = heads_w2[h] @ h + heads_b2[h]                # 64 -> 3 (padded), slice+concat -> 12

Host-side preprocessing folds BN (eval mode) into the conv weights, stacks the
6 head convs into a single [384, 64] matmul, and builds a block-diagonal
[12, 384] final conv that directly emits the channel-concatenated output.

Sharding: data-parallel over batch: core b handles ct_feat[b] ([256, 16384]).

Device kernel design (per core, N=16384 split into 32 tiles of F=512,
processed as 16 pairs):
  - bf16 matmuls (1 PE cycle/row, half the input DMA of fp32).
  - Every stationary is zero-padded to [128, 128] so every matmul runs with
    tile_size (128, 128): no PE array-mode switches, and stage-1/stage-3
    outputs pack two tiles into one PSUM bank (stage-1: y(jA) in partitions
    0-63, y(jB) in 64-127; stage-3: out(jA) in 0-11, out(jB) in 32-43).
  - PSUM budget: py x1 + ph x6 + po x1 = 8 banks.
  - The PE stream is software-pipelined: iteration p emits
    S1(p+1) | S2(p) | S3(p-2), so a matmul never waits on an eviction that
    was issued less than a full iteration (~3.4 us) earlier.
  - PSUM evictions (relu+bias / add-bias) are statically assigned to ACT
    and DVE only (GPSIMD cannot read PSUM): E1 ACT, E2 alternating DVE/ACT,
    E3 DVE -- 4 ops per engine per pair, both under the PE's 3.4 us.
"""

import numpy as np

BN_EPS = 1e-3
HEAD_CH = (3, 2, 1, 3, 2, 1)
B, CIN, N, CS, HN = 8, 256, 16384, 64, 6
COUT = sum(HEAD_CH)  # 12
NCORES = 8

MM_DTYPE = "bf16"

F = 512            # matmul free-dim tile (one fp32 PSUM bank)
PAIR = 2 * F       # two tiles processed per pipeline iteration
NPAIRS = N // PAIR  # 16

# packed stationary layout: 16 blocks of [128, 128] side by side:
#   blocks 0-3:  stage-1 (half, k): (A,k0) (A,k1) (B,k0) (B,k1)
#   blocks 4-9:  stage-2 (m, half): (0,A) (0,B) (1,A) (1,B) (2,A) (2,B)
#   blocks 10-15: stage-3 (k, half)
W1_BLK, W2_BLK, W3_BLK, NBLK = 0, 4, 10, 16
# bias tile [128, 5] f32: col0 [b1;b1], col1-3 b2 chunks, col4 b3 at rows
# {0-11, 32-43}
B1_COL, B2_COL, B3_COL, B_COLS = 0, 1, 4, 5

_CACHE: dict = {}


def _build_bass(mm_dtype: str, repeat: int = 1):
    import concourse.bacc as bacc
    import concourse.mybir as mybir
    from concourse.tile import TileContext

    f32 = mybir.dt.float32
    mdt = {"f32r": mybir.dt.float32r, "bf16": mybir.dt.bfloat16,
           "f16": mybir.dt.float16}[mm_dtype]
    AF = mybir.ActivationFunctionType

    nc = bacc.Bacc()
    x = nc.declare_dram_parameter("x", [CIN, N], mdt, isOutput=False)
    wp = nc.declare_dram_parameter("wp", [128, NBLK * 128], mdt, isOutput=False)
    bp = nc.declare_dram_parameter("bp", [128, B_COLS], f32, isOutput=False)
    out = nc.declare_dram_parameter("out", [COUT, N], f32, isOutput=True)

    with TileContext(nc) as tc:
        with (
            tc.tile_pool(name="const", bufs=1) as cpool,
            tc.tile_pool(name="xin", bufs=4) as xpool,
            tc.tile_pool(name="acts", bufs=3) as apool,
            tc.tile_pool(name="outs", bufs=2) as opool,
            tc.tile_pool(name="psum", bufs=2, space="PSUM") as ppool,
        ):
            wt = cpool.tile([128, NBLK * 128], mdt)
            # stage-1 stationaries first so S1(0) can start early; the
            # stage-2/3 blocks (wpB) follow the first two x tiles so the
            # serialized DMA engines deliver S1's inputs first.
            nc.scalar.dma_start(out=wt[:, 0:512], in_=wp[:, 0:512])
            bt = cpool.tile([128, B_COLS], f32)
            nc.scalar.dma_start(out=bt[:], in_=bp[:])

            wblk = [wt[:, i * 128 : (i + 1) * 128] for i in range(NBLK)]
            b1 = bt[:, B1_COL : B1_COL + 1]
            b2 = [bt[:, B2_COL + m : B2_COL + m + 1] for m in range(3)]
            b3 = bt[0:44, B3_COL : B3_COL + 1]

            # Warm-ups: make PE/ACT observe the const DMAs via single-wait
            # ops so no later matmul needs a second sync-wait slot.
            pw = ppool.tile([1, 1], f32, tag="po", bufs=1)
            wwu = (wt[:, 0:1].bitcast(f32) if mm_dtype == "f32r"
                   else wt[:, 0:1])
            nc.tensor.matmul(pw[:], wwu, wwu, start=True, stop=True)
            sw = apool.tile([1, 1], f32, tag="warm", bufs=1)
            nc.scalar.activation(sw[:], bt[0:1, 0:1], AF.Copy)
            # stage-2/3 stationaries: issued after the warm-ups so the first
            # x tiles win the serialized DMA engines
            nc.scalar.dma_start(out=wt[:, 512:], in_=wp[:, 512:])

            xr = x.rearrange("(k p) n -> p k n", p=128)

            import contextlib
            loop_cm = (tc.For_i(0, repeat,
                                hint_engines=(mybir.EngineType.PE,))
                       if repeat > 1 else contextlib.nullcontext())

            def relu_bias(eng, dst, src, bias_ap):
                if eng == "ACT":
                    nc.scalar.activation(dst, src, AF.Relu, bias=bias_ap)
                elif eng == "DVE":
                    nc.vector.tensor_scalar(dst, src, bias_ap, 0.0,
                                            mybir.AluOpType.add,
                                            mybir.AluOpType.max)
                else:
                    nc.gpsimd.tensor_scalar(dst, src, bias_ap, 0.0,
                                            mybir.AluOpType.add,
                                            mybir.AluOpType.max)

            def add_bias(eng, dst, src, bias_ap):
                if eng == "ACT":
                    nc.scalar.activation(dst, src, AF.Identity, bias=bias_ap)
                elif eng == "DVE":
                    nc.vector.tensor_scalar(dst, src, bias_ap, None,
                                            mybir.AluOpType.add)
                else:
                    nc.gpsimd.tensor_scalar(dst, src, bias_ap, None,
                                            mybir.AluOpType.add)

            with loop_cm:
                xt = {}      # pair -> x tile [128, 2, PAIR]
                ys = {}      # pair -> stage-1 output [128, F] (A|B packed)
                hs = {}      # pair -> list of 6 stage-2 outputs [128, F]

                def xdma(p):
                    xt[p] = xpool.tile([128, 2, PAIR], mdt, tag="xt", name=f"xt{p}")
                    nc.sync.dma_start(
                        out=xt[p][:],
                        in_=xr[:, :, p * PAIR : (p + 1) * PAIR])

                def s1(p):
                    py = ppool.tile([128, F], f32, tag="py", bufs=1)
                    xa = xt[p][:, :, 0:F]
                    xb = xt[p][:, :, F:PAIR]
                    nc.tensor.matmul(py[:], wblk[0], xa[:, 0], start=True,
                                     stop=False)
                    nc.tensor.matmul(py[:], wblk[1], xa[:, 1], start=False,
                                     stop=False)
                    nc.tensor.matmul(py[:], wblk[2], xb[:, 0], start=False,
                                     stop=False)
                    nc.tensor.matmul(py[:], wblk[3], xb[:, 1], start=False,
                                     stop=True)
                    ys[p] = apool.tile([128, F], mdt, tag="ys", bufs=3, name=f"ys{p}")
                    relu_bias("ACT", ys[p][:], py[:], b1)
                    del xt[p]

                def s2(p):
                    hs[p] = []
                    engs = ("DVE", "ACT")
                    for i in range(6):
                        m, half = i // 2, i % 2
                        ph = ppool.tile([128, F], f32, tag="ph", bufs=6)
                        nc.tensor.matmul(ph[:], wblk[W2_BLK + i], ys[p][:],
                                         start=True, stop=True)
                        h = apool.tile([128, F], mdt, tag="hs", bufs=18, name=f"hs{p}_{i}")
                        relu_bias(engs[i % 2], h[:], ph[:], b2[m])
                        hs[p].append(h)
                    del ys[p]

                def s3(p):
                    pob = ppool.tile([128, F], f32, tag="po", bufs=1)
                    for i in range(6):
                        # wblk (k, half) pairs with hs (m, half): m == k
                        nc.tensor.matmul(pob[:], wblk[W3_BLK + i],
                                         hs[p][i][:],
                                         start=(i == 0), stop=(i == 5))
                    ot = opool.tile([64, F], f32, tag="ot")
                    add_bias("DVE", ot[0:44, :], pob[0:44, :], b3)
                    del hs[p]
                    # SBUF APs honor only the leading partition dim, so the
                    # two 12-row groups go out as two plain DMAs; issued from
                    # SP (sync) -- a dma_start occupies its issuing engine's
                    # sequencer for ~600 ns, which would delay ACT evictions
                    c0 = p * PAIR
                    nc.sync.dma_start(out=out[:, c0 : c0 + F],
                                      in_=ot[0:COUT, :])
                    nc.sync.dma_start(out=out[:, c0 + F : c0 + PAIR],
                                      in_=ot[32 : 32 + COUT, :])

                # prologue
                xdma(0)
                xdma(1)
                xdma(2)
                s1(0)

                for p in range(NPAIRS):
                    if p + 3 < NPAIRS:
                        xdma(p + 3)
                    if p + 1 < NPAIRS:
                        s1(p + 1)
                    s2(p)
                    if p >= 2:
                        s3(p - 2)
                s3(NPAIRS - 2)
                s3(NPAIRS - 1)

    nc.finalize()
    _check_matmul_waits(nc)
    return nc


def _check_matmul_waits(nc):
    import concourse.mybir as mybir

    bad = []
    for f in nc.m.functions:
        for blk in f.blocks:
            for inst in blk.instructions:
                if isinstance(inst, mybir.InstMatmult) and inst.sync_info:
                    if len(inst.sync_info.on_wait) > 1:
                        bad.append((inst.name,
                                    [w.ant_name for w in inst.sync_info.on_wait]))
    if bad:
        raise RuntimeError(f"matmuls with >1 sync wait (walrus limit): {bad}")


def _get_nc(mm_dtype: str, repeat: int = 1):
    key = (mm_dtype, repeat)
    if key not in _CACHE:
        _CACHE[key] = _build_bass(mm_dtype, repeat)
    return _CACHE[key]


def _np_mm_dtype(mm_dtype: str):
    if mm_dtype == "bf16":
        import ml_dtypes
        return ml_dtypes.bfloat16
    if mm_dtype == "f16":
        return np.float16
    return np.float32  # f32r streams fp32 bits


def _fold_params(inputs, mm_dtype: str):
    """Fold BN into conv weights; pack into the on-device tile layouts."""
    f = lambda k: np.asarray(inputs[k], np.float32)

    inv1 = f("shared_gamma") / np.sqrt(f("shared_var") + BN_EPS)          # [64]
    W1 = f("shared_w") * inv1[:, None]                                    # [64, 256]
    b1v = f("shared_b") * inv1 + f("shared_beta") - f("shared_mean") * inv1

    inv2 = f("heads_gamma") / np.sqrt(f("heads_var") + BN_EPS)            # [6, 64]
    W2 = (f("heads_w1") * inv2[:, :, None]).reshape(HN * CS, CS)          # [384, 64]
    b2v = (f("heads_b1") * inv2 + f("heads_beta")
           - f("heads_mean") * inv2).reshape(HN * CS)                     # [384]

    hw2, hb2 = f("heads_w2"), f("heads_b2")
    W3 = np.zeros((COUT, HN * CS), np.float32)                            # [12, 384]
    b3v = np.zeros((COUT,), np.float32)
    r = 0
    for h, ch in enumerate(HEAD_CH):
        W3[r : r + ch, h * CS : (h + 1) * CS] = hw2[h, :ch, :]
        b3v[r : r + ch] = hb2[h, :ch]
        r += ch

    # 16 zero-padded [128, 128] stationary blocks (see module docstring)
    wpk = np.zeros((128, NBLK * 128), np.float32)
    for half in range(2):                      # stage-1: (A,k0)(A,k1)(B,k0)(B,k1)
        for k in range(2):
            blk = W1_BLK + half * 2 + k
            wpk[:, blk * 128 + half * 64 : blk * 128 + half * 64 + 64] = \
                W1[:, k * 128 : (k + 1) * 128].T
    for m in range(3):                         # stage-2: (m,A)(m,B)
        w2m = W2[m * 128 : (m + 1) * 128, :].T                            # [64, 128]
        for half in range(2):
            blk = W2_BLK + m * 2 + half
            wpk[half * 64 : half * 64 + 64, blk * 128 : (blk + 1) * 128] = w2m
    for k in range(3):                         # stage-3: (k,A)(k,B)
        w3k = W3[:, k * 128 : (k + 1) * 128].T                            # [128, 12]
        for half in range(2):
            blk = W3_BLK + k * 2 + half
            c0 = blk * 128 + half * 32
            wpk[:, c0 : c0 + COUT] = w3k

    bpk = np.zeros((128, B_COLS), np.float32)
    bpk[0:64, B1_COL] = b1v
    bpk[64:128, B1_COL] = b1v
    for m in range(3):
        bpk[:, B2_COL + m] = b2v[m * 128 : (m + 1) * 128]
    bpk[0:COUT, B3_COL] = b3v
    bpk[32 : 32 + COUT, B3_COL] = b3v

    wpk = wpk.astype(_np_mm_dtype(mm_dtype))
    return {"wp": wpk, "bp": bpk}


def _run(inputs, mm_dtype=MM_DTYPE, trace=False):
    from concourse.bass_utils import run_bass_kernel_spmd

    nc = _get_nc(mm_dtype)
    shared = _fold_params(inputs, mm_dtype)
    ct = np.asarray(inputs["ct_feat"], np.float32).astype(_np_mm_dtype(mm_dtype))
    in_maps = [
        {"x": np.ascontiguousarray(ct[b]), **shared} for b in range(B)
    ]
    res = run_bass_kernel_spmd(nc, in_maps, core_ids=list(range(NCORES)),
                               trace=trace)
    out = np.stack([res.results[b]["out"] for b in range(B)], axis=0)
    return out, res


def kernel(**inputs) -> np.ndarray:
    out, _ = _run(inputs)
    return out


# revision 17
# speedup vs baseline: 1.8419x; 1.2711x over previous
"""CenterFormer bbox head as a fused 3-stage matmul chain on 8 TRN2 cores.

Reference computation (per batch b, per proposal n):
  y = relu(BN(shared_w @ x + shared_b))            # 256 -> 64
  h = relu(BN(heads_w1[h] @ y + heads_b1[h]))      # 64 -> 64, 6 heads
  o = heads_w2[h] @ h + heads_b2[h]                # 64 -> 3 (padded), slice+concat -> 12

Host-side preprocessing folds BN (eval mode) into the conv weights, stacks the
6 head convs into a single [384, 64] matmul, and builds a block-diagonal
[12, 384] final conv that directly emits the channel-concatenated output.

Sharding: data-parallel over batch: core b handles ct_feat[b] ([256, 16384]).

Device kernel design (per core, N=16384 split into 32 tiles of F=512,
processed as 16 pairs):
  - bf16 matmuls (1 PE cycle/row, half the input DMA of fp32).
  - Every stationary is zero-padded to [128, 128] so every matmul runs with
    tile_size (128, 128): no PE array-mode switches, and stage-1/stage-3
    outputs pack two tiles into one PSUM bank (stage-1: y(jA) in partitions
    0-63, y(jB) in 64-127; stage-3: out(jA) in 0-11, out(jB) in 32-43).
  - PSUM budget: py x1 + ph x6 + po x1 = 8 banks.
  - The PE stream is software-pipelined: iteration p emits
    S1(p+1) | S2(p) | S3(p-2), so a matmul never waits on an eviction that
    was issued less than a full iteration (~3.4 us) earlier.
  - PSUM evictions (relu+bias / add-bias) are statically assigned to ACT
    and DVE only (GPSIMD cannot read PSUM): E1 ACT, E2 alternating DVE/ACT,
    E3 DVE -- 4 ops per engine per pair, both under the PE's 3.4 us.
"""

import numpy as np

BN_EPS = 1e-3
HEAD_CH = (3, 2, 1, 3, 2, 1)
B, CIN, N, CS, HN = 8, 256, 16384, 64, 6
COUT = sum(HEAD_CH)  # 12
NCORES = 8

MM_DTYPE = "bf16"

F = 512            # matmul free-dim tile (one fp32 PSUM bank)
PAIR = 2 * F       # two tiles processed per pipeline iteration
NPAIRS = N // PAIR  # 16

# packed stationary layout: 13 blocks of [128, 128] side by side:
#   blocks 0-3:  stage-1 (half, k): (A,k0) (A,k1) (B,k0) (B,k1)
#   blocks 4-6:  stage-2 m: W2T_m duplicated in both partition halves
#   blocks 7-12: stage-3 (k, half)
W1_BLK, W2_BLK, W3_BLK, NBLK = 0, 4, 7, 13
# bias tile [128, 5] f32: col0 [b1;b1], col1-3 b2 chunks, col4 b3 at rows
# {0-11, 32-43}
B1_COL, B2_COL, B3_COL, B_COLS = 0, 1, 4, 5

_CACHE: dict = {}


def _build_bass(mm_dtype: str, repeat: int = 1):
    import concourse.bacc as bacc
    import concourse.mybir as mybir
    from concourse.tile import TileContext

    f32 = mybir.dt.float32
    mdt = {"f32r": mybir.dt.float32r, "bf16": mybir.dt.bfloat16,
           "f16": mybir.dt.float16}[mm_dtype]
    AF = mybir.ActivationFunctionType

    nc = bacc.Bacc()
    x = nc.declare_dram_parameter("x", [CIN, N], mdt, isOutput=False)
    wp = nc.declare_dram_parameter("wp", [128, NBLK * 128], mdt, isOutput=False)
    bp = nc.declare_dram_parameter("bp", [128, B_COLS], f32, isOutput=False)
    out = nc.declare_dram_parameter("out", [COUT, N], f32, isOutput=True)

    with TileContext(nc) as tc:
        with (
            tc.tile_pool(name="const", bufs=1) as cpool,
            tc.tile_pool(name="xin", bufs=4) as xpool,
            tc.tile_pool(name="acts", bufs=3) as apool,
            tc.tile_pool(name="outs", bufs=2) as opool,
            tc.tile_pool(name="psum", bufs=2, space="PSUM") as ppool,
        ):
            wt = cpool.tile([128, NBLK * 128], mdt)
            # stage-1 stationaries first so S1(0) can start early; the
            # stage-2/3 blocks (wpB) follow the first two x tiles so the
            # serialized DMA engines deliver S1's inputs first.
            nc.scalar.dma_start(out=wt[:, 0:512], in_=wp[:, 0:512])
            bt = cpool.tile([128, B_COLS], f32)
            nc.scalar.dma_start(out=bt[:], in_=bp[:])

            wblk = [wt[:, i * 128 : (i + 1) * 128] for i in range(NBLK)]
            b1 = bt[:, B1_COL : B1_COL + 1]
            b2 = [bt[:, B2_COL + m : B2_COL + m + 1] for m in range(3)]
            b3 = bt[0:44, B3_COL : B3_COL + 1]

            # Warm-ups: make PE/ACT observe the const DMAs via single-wait
            # ops so no later matmul needs a second sync-wait slot.
            pw = ppool.tile([1, 1], f32, tag="po", bufs=1)
            wwu = (wt[:, 0:1].bitcast(f32) if mm_dtype == "f32r"
                   else wt[:, 0:1])
            nc.tensor.matmul(pw[:], wwu, wwu, start=True, stop=True)
            sw = apool.tile([1, 1], f32, tag="warm", bufs=1)
            nc.scalar.activation(sw[:], bt[0:1, 0:1], AF.Copy)
            # stage-2/3 stationaries: issued after the warm-ups so the first
            # x tiles win the serialized DMA engines
            nc.scalar.dma_start(out=wt[:, 512:], in_=wp[:, 512:])

            xr = x.rearrange("(k p) n -> p k n", p=128)

            import contextlib
            loop_cm = (tc.For_i(0, repeat,
                                hint_engines=(mybir.EngineType.PE,))
                       if repeat > 1 else contextlib.nullcontext())

            def relu_bias(eng, dst, src, bias_ap):
                if eng == "ACT":
                    nc.scalar.activation(dst, src, AF.Relu, bias=bias_ap)
                elif eng == "DVE":
                    nc.vector.tensor_scalar(dst, src, bias_ap, 0.0,
                                            mybir.AluOpType.add,
                                            mybir.AluOpType.max)
                else:
                    nc.gpsimd.tensor_scalar(dst, src, bias_ap, 0.0,
                                            mybir.AluOpType.add,
                                            mybir.AluOpType.max)

            def add_bias(eng, dst, src, bias_ap):
                if eng == "ACT":
                    nc.scalar.activation(dst, src, AF.Identity, bias=bias_ap)
                elif eng == "DVE":
                    nc.vector.tensor_scalar(dst, src, bias_ap, None,
                                            mybir.AluOpType.add)
                else:
                    nc.gpsimd.tensor_scalar(dst, src, bias_ap, None,
                                            mybir.AluOpType.add)

            with loop_cm:
                xt = {}      # pair -> x tile [128, 2, PAIR]
                ys = {}      # pair -> stage-1 output [128, F] (A|B packed)
                hs = {}      # pair -> list of 6 stage-2 outputs [128, F]

                def xdma(p):
                    xt[p] = xpool.tile([128, 2, PAIR], mdt, tag="xt", name=f"xt{p}")
                    nc.sync.dma_start(
                        out=xt[p][:],
                        in_=xr[:, :, p * PAIR : (p + 1) * PAIR])

                def s1(p):
                    py = ppool.tile([128, F], f32, tag="py", bufs=1)
                    xa = xt[p][:, :, 0:F]
                    xb = xt[p][:, :, F:PAIR]
                    nc.tensor.matmul(py[:], wblk[0], xa[:, 0], start=True,
                                     stop=False)
                    nc.tensor.matmul(py[:], wblk[1], xa[:, 1], start=False,
                                     stop=False)
                    nc.tensor.matmul(py[:], wblk[2], xb[:, 0], start=False,
                                     stop=False)
                    nc.tensor.matmul(py[:], wblk[3], xb[:, 1], start=False,
                                     stop=True)
                    ys[p] = apool.tile([128, F], mdt, tag="ys", bufs=3, name=f"ys{p}")
                    relu_bias("ACT", ys[p][:], py[:], b1)
                    del xt[p]

                def s2(p):
                    # K=64 matmuls on alternating 64-row PE tiles (0,0)/(64,0)
                    # overlap on HW (~1.7x measured): W2T_m is duplicated in
                    # both partition halves of its block; half A contracts
                    # ys[0:64] (= y of tile jA), half B contracts ys[64:128]
                    hs[p] = []
                    engs = ("DVE", "ACT")
                    for i in range(6):
                        m, half = i // 2, i % 2
                        r0 = 64 * half
                        ph = ppool.tile([128, F], f32, tag="ph", bufs=6)
                        nc.tensor.matmul(ph[:],
                                         wblk[W2_BLK + m][r0 : r0 + 64, :],
                                         ys[p][r0 : r0 + 64, :],
                                         start=True, stop=True)
                        h = apool.tile([128, F], mdt, tag="hs", bufs=18, name=f"hs{p}_{i}")
                        relu_bias(engs[i % 2], h[:], ph[:], b2[m])
                        hs[p].append(h)
                    del ys[p]

                def s3(p):
                    pob = ppool.tile([128, F], f32, tag="po", bufs=1)
                    for i in range(6):
                        # wblk (k, half) pairs with hs (m, half): m == k
                        nc.tensor.matmul(pob[:], wblk[W3_BLK + i],
                                         hs[p][i][:],
                                         start=(i == 0), stop=(i == 5))
                    ot = opool.tile([64, F], f32, tag="ot")
                    add_bias("DVE", ot[0:44, :], pob[0:44, :], b3)
                    del hs[p]
                    # SBUF APs honor only the leading partition dim, so the
                    # two 12-row groups go out as two plain DMAs; issued from
                    # SP (sync) -- a dma_start occupies its issuing engine's
                    # sequencer for ~600 ns, which would delay ACT evictions
                    c0 = p * PAIR
                    nc.sync.dma_start(out=out[:, c0 : c0 + F],
                                      in_=ot[0:COUT, :])
                    nc.sync.dma_start(out=out[:, c0 + F : c0 + PAIR],
                                      in_=ot[32 : 32 + COUT, :])

                # prologue
                xdma(0)
                xdma(1)
                xdma(2)
                s1(0)

                for p in range(NPAIRS):
                    if p + 3 < NPAIRS:
                        xdma(p + 3)
                    if p + 1 < NPAIRS:
                        s1(p + 1)
                    s2(p)
                    if p >= 2:
                        s3(p - 2)
                s3(NPAIRS - 2)
                s3(NPAIRS - 1)

    nc.finalize()
    _check_matmul_waits(nc)
    return nc


def _check_matmul_waits(nc):
    import concourse.mybir as mybir

    bad = []
    for f in nc.m.functions:
        for blk in f.blocks:
            for inst in blk.instructions:
                if isinstance(inst, mybir.InstMatmult) and inst.sync_info:
                    if len(inst.sync_info.on_wait) > 1:
                        bad.append((inst.name,
                                    [w.ant_name for w in inst.sync_info.on_wait]))
    if bad:
        raise RuntimeError(f"matmuls with >1 sync wait (walrus limit): {bad}")


def _get_nc(mm_dtype: str, repeat: int = 1):
    key = (mm_dtype, repeat)
    if key not in _CACHE:
        _CACHE[key] = _build_bass(mm_dtype, repeat)
    return _CACHE[key]


def _np_mm_dtype(mm_dtype: str):
    if mm_dtype == "bf16":
        import ml_dtypes
        return ml_dtypes.bfloat16
    if mm_dtype == "f16":
        return np.float16
    return np.float32  # f32r streams fp32 bits


def _fold_params(inputs, mm_dtype: str):
    """Fold BN into conv weights; pack into the on-device tile layouts."""
    f = lambda k: np.asarray(inputs[k], np.float32)

    inv1 = f("shared_gamma") / np.sqrt(f("shared_var") + BN_EPS)          # [64]
    W1 = f("shared_w") * inv1[:, None]                                    # [64, 256]
    b1v = f("shared_b") * inv1 + f("shared_beta") - f("shared_mean") * inv1

    inv2 = f("heads_gamma") / np.sqrt(f("heads_var") + BN_EPS)            # [6, 64]
    W2 = (f("heads_w1") * inv2[:, :, None]).reshape(HN * CS, CS)          # [384, 64]
    b2v = (f("heads_b1") * inv2 + f("heads_beta")
           - f("heads_mean") * inv2).reshape(HN * CS)                     # [384]

    hw2, hb2 = f("heads_w2"), f("heads_b2")
    W3 = np.zeros((COUT, HN * CS), np.float32)                            # [12, 384]
    b3v = np.zeros((COUT,), np.float32)
    r = 0
    for h, ch in enumerate(HEAD_CH):
        W3[r : r + ch, h * CS : (h + 1) * CS] = hw2[h, :ch, :]
        b3v[r : r + ch] = hb2[h, :ch]
        r += ch

    # 16 zero-padded [128, 128] stationary blocks (see module docstring)
    wpk = np.zeros((128, NBLK * 128), np.float32)
    for half in range(2):                      # stage-1: (A,k0)(A,k1)(B,k0)(B,k1)
        for k in range(2):
            blk = W1_BLK + half * 2 + k
            wpk[:, blk * 128 + half * 64 : blk * 128 + half * 64 + 64] = \
                W1[:, k * 128 : (k + 1) * 128].T
    for m in range(3):                         # stage-2: W2T_m in both halves
        w2m = W2[m * 128 : (m + 1) * 128, :].T                            # [64, 128]
        blk = W2_BLK + m
        wpk[0:64, blk * 128 : (blk + 1) * 128] = w2m
        wpk[64:128, blk * 128 : (blk + 1) * 128] = w2m
    for k in range(3):                         # stage-3: (k,A)(k,B)
        w3k = W3[:, k * 128 : (k + 1) * 128].T                            # [128, 12]
        for half in range(2):
            blk = W3_BLK + k * 2 + half
            c0 = blk * 128 + half * 32
            wpk[:, c0 : c0 + COUT] = w3k

    bpk = np.zeros((128, B_COLS), np.float32)
    bpk[0:64, B1_COL] = b1v
    bpk[64:128, B1_COL] = b1v
    for m in range(3):
        bpk[:, B2_COL + m] = b2v[m * 128 : (m + 1) * 128]
    bpk[0:COUT, B3_COL] = b3v
    bpk[32 : 32 + COUT, B3_COL] = b3v

    wpk = wpk.astype(_np_mm_dtype(mm_dtype))
    return {"wp": wpk, "bp": bpk}


def _run(inputs, mm_dtype=MM_DTYPE, trace=False):
    from concourse.bass_utils import run_bass_kernel_spmd

    nc = _get_nc(mm_dtype)
    shared = _fold_params(inputs, mm_dtype)
    ct = np.asarray(inputs["ct_feat"], np.float32).astype(_np_mm_dtype(mm_dtype))
    in_maps = [
        {"x": np.ascontiguousarray(ct[b]), **shared} for b in range(B)
    ]
    res = run_bass_kernel_spmd(nc, in_maps, core_ids=list(range(NCORES)),
                               trace=trace)
    out = np.stack([res.results[b]["out"] for b in range(B)], axis=0)
    return out, res


def kernel(**inputs) -> np.ndarray:
    out, _ = _run(inputs)
    return out


# revision 18
# speedup vs baseline: 4.2964x; 2.3325x over previous
"""CenterFormer bbox head as a fused 3-stage matmul chain on 8 TRN2 cores.

Reference computation (per batch b, per proposal n):
  y = relu(BN(shared_w @ x + shared_b))            # 256 -> 64
  h = relu(BN(heads_w1[h] @ y + heads_b1[h]))      # 64 -> 64, 6 heads
  o = heads_w2[h] @ h + heads_b2[h]                # 64 -> 3 (padded), slice+concat -> 12

Host-side preprocessing folds BN (eval mode) into the conv weights, stacks the
6 head convs into a single [384, 64] matmul, and builds a block-diagonal
[12, 384] final conv that directly emits the channel-concatenated output.

Sharding: data-parallel over batch: core b handles ct_feat[b] ([256, 16384]).

Device kernel design (per core, N=16384 split into 32 tiles of F=512,
processed as 16 pairs):
  - bf16 matmuls (1 PE cycle/row, half the input DMA of fp32).
  - Every stationary is zero-padded to [128, 128] so every matmul runs with
    tile_size (128, 128): no PE array-mode switches, and stage-1/stage-3
    outputs pack two tiles into one PSUM bank (stage-1: y(jA) in partitions
    0-63, y(jB) in 64-127; stage-3: out(jA) in 0-11, out(jB) in 32-43).
  - PSUM budget: py x1 + ph x6 + po x1 = 8 banks.
  - The PE stream is software-pipelined: iteration p emits
    S1(p+1) | S2(p) | S3(p-2), so a matmul never waits on an eviction that
    was issued less than a full iteration (~3.4 us) earlier.
  - PSUM evictions (relu+bias / add-bias) are statically assigned to ACT
    and DVE only (GPSIMD cannot read PSUM): E1 ACT, E2 alternating DVE/ACT,
    E3 DVE -- 4 ops per engine per pair, both under the PE's 3.4 us.
"""

import numpy as np

BN_EPS = 1e-3
HEAD_CH = (3, 2, 1, 3, 2, 1)
B, CIN, N, CS, HN = 8, 256, 16384, 64, 6
COUT = sum(HEAD_CH)  # 12
NCORES = 8

MM_DTYPE = "bf16"

F = 512            # matmul free-dim tile (one fp32 PSUM bank)
PAIR = 2 * F       # two tiles processed per pipeline iteration
NPAIRS = N // PAIR  # 16

# packed stationary layout (columns of the [128, 608] weight tile):
#   cols 0-127:   stage-1: W1T k-chunks, [128, 64] each (shared by A/B halves
#                 via output column tiles)
#   cols 128-511: stage-2: W2T_m [128, 128], duplicated in both partition
#                 halves (row tiles)
#   cols 512-607: stage-3: W3T k-chunks padded to [128, 32] (shared by A/B
#                 via output column tiles)
W1_OFF, W2_OFF, W3_OFF, W_COLS = 0, 128, 512, 608
# bias tile [128, 5] f32: col0 [b1;b1], col1-3 b2 chunks, col4 b3 at rows
# {0-11, 32-43}
B1_COL, B2_COL, B3_COL, B_COLS = 0, 1, 4, 5

_CACHE: dict = {}


def _build_bass(mm_dtype: str, repeat: int = 1):
    import concourse.bacc as bacc
    import concourse.mybir as mybir
    from concourse.tile import TileContext

    f32 = mybir.dt.float32
    mdt = {"f32r": mybir.dt.float32r, "bf16": mybir.dt.bfloat16,
           "f16": mybir.dt.float16}[mm_dtype]
    AF = mybir.ActivationFunctionType

    nc = bacc.Bacc()
    x = nc.declare_dram_parameter("x", [CIN, N], mdt, isOutput=False)
    wp = nc.declare_dram_parameter("wp", [128, W_COLS], mdt, isOutput=False)
    bp = nc.declare_dram_parameter("bp", [128, B_COLS], f32, isOutput=False)
    out = nc.declare_dram_parameter("out", [COUT, N], f32, isOutput=True)

    with TileContext(nc) as tc:
        with (
            tc.tile_pool(name="const", bufs=1) as cpool,
            tc.tile_pool(name="xin", bufs=4) as xpool,
            tc.tile_pool(name="acts", bufs=3) as apool,
            tc.tile_pool(name="outs", bufs=2) as opool,
            tc.tile_pool(name="psum", bufs=2, space="PSUM") as ppool,
        ):
            wt = cpool.tile([128, W_COLS], mdt)
            # stage-1 stationaries first so S1(0) can start early; the
            # stage-2/3 blocks (wpB) follow the first two x tiles so the
            # serialized DMA engines deliver S1's inputs first.
            nc.scalar.dma_start(out=wt[:, 0:W2_OFF], in_=wp[:, 0:W2_OFF])
            bt = cpool.tile([128, B_COLS], f32)
            nc.scalar.dma_start(out=bt[:], in_=bp[:])

            w1 = [wt[:, W1_OFF + k * 64 : W1_OFF + (k + 1) * 64]
                  for k in range(2)]
            w2 = [wt[:, W2_OFF + m * 128 : W2_OFF + (m + 1) * 128]
                  for m in range(3)]
            w3 = [wt[:, W3_OFF + k * 32 : W3_OFF + (k + 1) * 32]
                  for k in range(3)]
            b1 = bt[:, B1_COL : B1_COL + 1]
            b2 = [bt[:, B2_COL + m : B2_COL + m + 1] for m in range(3)]
            b3 = bt[0:44, B3_COL : B3_COL + 1]

            # Warm-ups: make PE/ACT observe the const DMAs via single-wait
            # ops so no later matmul needs a second sync-wait slot.
            pw = ppool.tile([1, 1], f32, tag="po", bufs=1)
            wwu = (wt[:, 0:1].bitcast(f32) if mm_dtype == "f32r"
                   else wt[:, 0:1])
            nc.tensor.matmul(pw[:], wwu, wwu, start=True, stop=True)
            sw = apool.tile([1, 1], f32, tag="warm", bufs=1)
            nc.scalar.activation(sw[:], bt[0:1, 0:1], AF.Copy)
            # stage-2/3 stationaries: issued after the warm-ups so the first
            # x tiles win the serialized DMA engines
            nc.scalar.dma_start(out=wt[:, W2_OFF:], in_=wp[:, W2_OFF:])

            xr = x.rearrange("(k p) n -> p k n", p=128)

            import contextlib
            loop_cm = (tc.For_i(0, repeat,
                                hint_engines=(mybir.EngineType.PE,))
                       if repeat > 1 else contextlib.nullcontext())

            def relu_bias(eng, dst, src, bias_ap):
                if eng == "ACT":
                    nc.scalar.activation(dst, src, AF.Relu, bias=bias_ap)
                elif eng == "DVE":
                    nc.vector.tensor_scalar(dst, src, bias_ap, 0.0,
                                            mybir.AluOpType.add,
                                            mybir.AluOpType.max)
                else:
                    nc.gpsimd.tensor_scalar(dst, src, bias_ap, 0.0,
                                            mybir.AluOpType.add,
                                            mybir.AluOpType.max)

            def add_bias(eng, dst, src, bias_ap):
                if eng == "ACT":
                    nc.scalar.activation(dst, src, AF.Identity, bias=bias_ap)
                elif eng == "DVE":
                    nc.vector.tensor_scalar(dst, src, bias_ap, None,
                                            mybir.AluOpType.add)
                else:
                    nc.gpsimd.tensor_scalar(dst, src, bias_ap, None,
                                            mybir.AluOpType.add)

            with loop_cm:
                xt = {}      # pair -> x tile [128, 2, PAIR]
                ys = {}      # pair -> stage-1 output [128, F] (A|B packed)
                hs = {}      # pair -> list of 6 stage-2 outputs [128, F]

                def xdma(p):
                    xt[p] = xpool.tile([128, 2, PAIR], mdt, tag="xt", name=f"xt{p}")
                    nc.sync.dma_start(
                        out=xt[p][:],
                        in_=xr[:, :, p * PAIR : (p + 1) * PAIR])

                def s1(p):
                    # column tiles (128K, 64M): half A accumulates into py
                    # partitions 0-63, half B into 64-127; alternating tile
                    # positions (0,0)/(0,64) overlap on the PE
                    py = ppool.tile([128, F], f32, tag="py", bufs=1)
                    xa = xt[p][:, :, 0:F]
                    xb = xt[p][:, :, F:PAIR]
                    nc.tensor.matmul(py[0:64, :], w1[0], xa[:, 0],
                                     start=True, stop=False)
                    nc.tensor.matmul(py[64:128, :], w1[0], xb[:, 0],
                                     start=True, stop=False)
                    nc.tensor.matmul(py[0:64, :], w1[1], xa[:, 1],
                                     start=False, stop=True)
                    nc.tensor.matmul(py[64:128, :], w1[1], xb[:, 1],
                                     start=False, stop=True)
                    ys[p] = apool.tile([128, F], mdt, tag="ys", bufs=3, name=f"ys{p}")
                    relu_bias("ACT", ys[p][:], py[:], b1)
                    del xt[p]

                def s2(p):
                    # K=64 matmuls on alternating 64-row PE tiles (0,0)/(64,0)
                    # overlap on HW (~1.7x measured): W2T_m is duplicated in
                    # both partition halves of its block; half A contracts
                    # ys[0:64] (= y of tile jA), half B contracts ys[64:128]
                    hs[p] = []
                    engs = ("DVE", "ACT")
                    for i in range(6):
                        m, half = i // 2, i % 2
                        r0 = 64 * half
                        ph = ppool.tile([128, F], f32, tag="ph", bufs=6)
                        nc.tensor.matmul(ph[:],
                                         w2[m][r0 : r0 + 64, :],
                                         ys[p][r0 : r0 + 64, :],
                                         start=True, stop=True)
                        h = apool.tile([128, F], mdt, tag="hs", bufs=18, name=f"hs{p}_{i}")
                        relu_bias(engs[i % 2], h[:], ph[:], b2[m])
                        hs[p].append(h)
                    del ys[p]

                def s3(p):
                    # column tiles (128K, 32M): half A accumulates into pob
                    # partitions 0-31 (12 real + zero-pad), half B into
                    # 32-63; alternating positions (0,0)/(0,32) overlap
                    pob = ppool.tile([128, F], f32, tag="po", bufs=1)
                    for i in range(6):
                        k, half = i // 2, i % 2
                        c0 = 32 * half
                        nc.tensor.matmul(pob[c0 : c0 + 32, :], w3[k],
                                         hs[p][i][:],
                                         start=(i < 2), stop=(i >= 4))
                    ot = opool.tile([64, F], f32, tag="ot")
                    add_bias("DVE", ot[0:44, :], pob[0:44, :], b3)
                    del hs[p]
                    # SBUF APs honor only the leading partition dim, so the
                    # two 12-row groups go out as two plain DMAs; issued from
                    # SP (sync) -- a dma_start occupies its issuing engine's
                    # sequencer for ~600 ns, which would delay ACT evictions
                    c0 = p * PAIR
                    nc.sync.dma_start(out=out[:, c0 : c0 + F],
                                      in_=ot[0:COUT, :])
                    nc.sync.dma_start(out=out[:, c0 + F : c0 + PAIR],
                                      in_=ot[32 : 32 + COUT, :])

                # prologue
                xdma(0)
                xdma(1)
                xdma(2)
                s1(0)

                for p in range(NPAIRS):
                    if p + 3 < NPAIRS:
                        xdma(p + 3)
                    if p + 1 < NPAIRS:
                        s1(p + 1)
                    s2(p)
                    if p >= 2:
                        s3(p - 2)
                s3(NPAIRS - 2)
                s3(NPAIRS - 1)

    nc.finalize()
    _check_matmul_waits(nc)
    return nc


def _check_matmul_waits(nc):
    import concourse.mybir as mybir

    bad = []
    for f in nc.m.functions:
        for blk in f.blocks:
            for inst in blk.instructions:
                if isinstance(inst, mybir.InstMatmult) and inst.sync_info:
                    if len(inst.sync_info.on_wait) > 1:
                        bad.append((inst.name,
                                    [w.ant_name for w in inst.sync_info.on_wait]))
    if bad:
        raise RuntimeError(f"matmuls with >1 sync wait (walrus limit): {bad}")


def _get_nc(mm_dtype: str, repeat: int = 1):
    key = (mm_dtype, repeat)
    if key not in _CACHE:
        _CACHE[key] = _build_bass(mm_dtype, repeat)
    return _CACHE[key]


def _np_mm_dtype(mm_dtype: str):
    if mm_dtype == "bf16":
        import ml_dtypes
        return ml_dtypes.bfloat16
    if mm_dtype == "f16":
        return np.float16
    return np.float32  # f32r streams fp32 bits


def _fold_params(inputs, mm_dtype: str):
    """Fold BN into conv weights; pack into the on-device tile layouts."""
    f = lambda k: np.asarray(inputs[k], np.float32)

    inv1 = f("shared_gamma") / np.sqrt(f("shared_var") + BN_EPS)          # [64]
    W1 = f("shared_w") * inv1[:, None]                                    # [64, 256]
    b1v = f("shared_b") * inv1 + f("shared_beta") - f("shared_mean") * inv1

    inv2 = f("heads_gamma") / np.sqrt(f("heads_var") + BN_EPS)            # [6, 64]
    W2 = (f("heads_w1") * inv2[:, :, None]).reshape(HN * CS, CS)          # [384, 64]
    b2v = (f("heads_b1") * inv2 + f("heads_beta")
           - f("heads_mean") * inv2).reshape(HN * CS)                     # [384]

    hw2, hb2 = f("heads_w2"), f("heads_b2")
    W3 = np.zeros((COUT, HN * CS), np.float32)                            # [12, 384]
    b3v = np.zeros((COUT,), np.float32)
    r = 0
    for h, ch in enumerate(HEAD_CH):
        W3[r : r + ch, h * CS : (h + 1) * CS] = hw2[h, :ch, :]
        b3v[r : r + ch] = hb2[h, :ch]
        r += ch

    # packed stationaries (see module docstring for the layout)
    wpk = np.zeros((128, W_COLS), np.float32)
    for k in range(2):                         # stage-1 k-chunks [128, 64]
        wpk[:, W1_OFF + k * 64 : W1_OFF + (k + 1) * 64] = \
            W1[:, k * 128 : (k + 1) * 128].T
    for m in range(3):                         # stage-2: W2T_m in both halves
        w2m = W2[m * 128 : (m + 1) * 128, :].T                            # [64, 128]
        wpk[0:64, W2_OFF + m * 128 : W2_OFF + (m + 1) * 128] = w2m
        wpk[64:128, W2_OFF + m * 128 : W2_OFF + (m + 1) * 128] = w2m
    for k in range(3):                         # stage-3 k-chunks [128, 32]
        wpk[:, W3_OFF + k * 32 : W3_OFF + k * 32 + COUT] = \
            W3[:, k * 128 : (k + 1) * 128].T

    bpk = np.zeros((128, B_COLS), np.float32)
    bpk[0:64, B1_COL] = b1v
    bpk[64:128, B1_COL] = b1v
    for m in range(3):
        bpk[:, B2_COL + m] = b2v[m * 128 : (m + 1) * 128]
    bpk[0:COUT, B3_COL] = b3v
    bpk[32 : 32 + COUT, B3_COL] = b3v

    wpk = wpk.astype(_np_mm_dtype(mm_dtype))
    return {"wp": wpk, "bp": bpk}


def _run(inputs, mm_dtype=MM_DTYPE, trace=False):
    from concourse.bass_utils import run_bass_kernel_spmd

    nc = _get_nc(mm_dtype)
    shared = _fold_params(inputs, mm_dtype)
    ct = np.asarray(inputs["ct_feat"], np.float32).astype(_np_mm_dtype(mm_dtype))
    in_maps = [
        {"x": np.ascontiguousarray(ct[b]), **shared} for b in range(B)
    ]
    res = run_bass_kernel_spmd(nc, in_maps, core_ids=list(range(NCORES)),
                               trace=trace)
    out = np.stack([res.results[b]["out"] for b in range(B)], axis=0)
    return out, res


def kernel(**inputs) -> np.ndarray:
    out, _ = _run(inputs)
    return out
